# revision 16
# baseline (speedup 1.0000x reference)
"""Trainium2 Bass kernel for nn_PoolingLayer (target-attention pooling layer).

Computation (per batch b):
  K = prelu(x @ W_k.T), V = prelu(x @ W_v.T)           x: [S, D]
  Q = prelu(e @ W_q.T);  Qt = W_kernel @ Q
  score[s] = K[s] . Qt / sqrt(A);  masked softmax over s -> attn
  out = sum_s attn[s] * V[s];  ffn_out = prelu(out @ ffn_W.T + ffn_b)
Returns (ffn_out [B, D], attn [B, 1, S]).

Sharding: pure data-parallel over batch across 8 NeuronCores (256 b/core).
On-chip layout: per b-tile of 128 batches; x streamed in via DMA-transpose
(bf16) as xT [d, m] tiles; K kept transposed [a, m] for per-b score matmuls;
V kept row-major in per-(b, s-chunk) blocks for per-b attn.V matmuls, with
s-chunks [0:128] and [72:200] (the 56-column overlap of chunk 1 is zeroed in
the transposed-attention operand so nothing is double counted).
"""

import numpy as np
import ml_dtypes
from contextlib import ExitStack

B, S, D, A = 2048, 200, 256, 128
DI = 64                 # target item embedding dim
N_CORES = 8
BC = B // N_CORES       # 256 batches per core
BT = 128                # b-tile (partition) size
NBT = BC // BT          # b-tiles per core
NPAIR = BT // 2         # batch pairs per b-tile (xT loaded per pair: 400 rows)
SCALE = float(A) ** 0.5
SC_OFF = (0, 128)       # s-chunk starts: [0:128], [128:200]
SC_LEN = (128, S - 128) # s-chunk lengths: 128, 72

BF16 = ml_dtypes.bfloat16

_cache = {}


# ---------------------------------------------------------------------------
# walrus workaround: this build accepts only 1 sem-wait on the Tile kernel-tail
# Drain; split the waits across single-wait sync NoOps.
def _install_tile_patch():
    import bass_rust
    import concourse.tile as _tile
    from concourse.vector_clock import ScopedClock

    if getattr(_tile.TileContext, "_drain_patched", False):
        return

    def _patched(self, tick_clock, wait_clock):
        nc = self.nc
        drain_inst = nc.sync.drain()
        wait_clock.add_sem_waits(
            drain_inst.ins, ScopedClock({None: tick_clock.global_clock})
        )
        si = drain_inst.ins.sync_info
        if si is not None and len(si.on_wait) > 1:
            waits = list(si.on_wait)
            si.on_wait = waits[:1]
            for w in waits[1:]:
                n = nc.sync.nop(nofuse=True)
                n.ins.sync_info = bass_rust.SyncInfo(on_update=[], on_wait=[w])
        nc.all_engine_barrier()
        assert self.sems is not None
        popped = nc._tile_sem_poison_stack.pop()
        assert popped is self._sem_poison
        nc.clear_and_free_semaphores(list(self.sems.allocated().values()))
        nc.all_engine_barrier()

    _tile.TileContext._drain_and_barrier = _patched
    _tile.TileContext._drain_patched = True

    # Global workaround: walrus accepts at most ONE sem-wait per instruction.
    # Rewrite the serialized BIR: move extra waits onto same-engine NoOps
    # inserted immediately before the over-subscribed instruction.
    import orjson
    import concourse.bass as _bass

    if getattr(_bass.Bass, "_json_wait_patched", False):
        return
    _orig_to_json_bytes = _bass.Bass.to_json_bytes

    def _split_waits_json(self):
        raw = _orig_to_json_bytes(self)
        bir = orjson.loads(raw)
        n = [0]

        def fix_block(bb):
            out = []
            for ins in bb.get("instructions", []):
                si = ins.get("sync_info") or {}
                w = si.get("on_wait") or []
                if len(w) > 1:
                    for extra in w[:-1]:
                        n[0] += 1
                        out.append({
                            "debug": ins.get("debug", 0),
                            "engine": ins["engine"],
                            "ins": [], "outs": [],
                            "name": f"{ins['name']}-wsplit{n[0]}",
                            "opcode": "NoOp",
                            "sync_info": {"on_update": [], "on_wait": [extra]},
                        })
                    si["on_wait"] = [w[-1]]
                out.append(ins)
            bb["instructions"] = out
            for sub in bb.get("blocks", []):
                fix_block(sub)

        for fn in bir.get("functions", []):
            for bb in fn.get("blocks", []):
                fix_block(bb)
        return orjson.dumps(bir)

    _bass.Bass.to_json_bytes = _split_waits_json
    _bass.Bass._json_wait_patched = True


# ---------------------------------------------------------------------------
def _build(alpha: float):
    import concourse.bass as bass
    import concourse.mybir as mybir
    import concourse.tile as tile

    _install_tile_patch()
    dt = mybir.dt
    AF = mybir.ActivationFunctionType
    OP = mybir.AluOpType

    nc = bass.Bass()
    xT = nc.declare_dram_parameter("xT", [D, BC * S], dt.bfloat16, isOutput=False)
    neg = nc.declare_dram_parameter("neg", [BC, S], dt.float32, isOutput=False)
    eb = nc.declare_dram_parameter("eb", [BC, DI], dt.bfloat16, isOutput=False)
    wkT = nc.declare_dram_parameter("wkT", [D, A], dt.bfloat16, isOutput=False)
    wvT = nc.declare_dram_parameter("wvT", [D, A], dt.bfloat16, isOutput=False)
    wqT = nc.declare_dram_parameter("wqT", [DI, A], dt.bfloat16, isOutput=False)
    wkerT = nc.declare_dram_parameter("wkerT", [A, A], dt.bfloat16, isOutput=False)
    fwT = nc.declare_dram_parameter("fwT", [A, D], dt.bfloat16, isOutput=False)
    fb = nc.declare_dram_parameter("fb", [1, D], dt.float32, isOutput=False)
    idb = nc.declare_dram_parameter("idb", [128, 128], dt.bfloat16, isOutput=False)
    idf = nc.declare_dram_parameter("idf", [128, 128], dt.float32, isOutput=False)
    ffn_o = nc.declare_dram_parameter("ffn_o", [BC, D], dt.float32, isOutput=True)
    attn_o = nc.declare_dram_parameter("attn_o", [BC, S], dt.float32, isOutput=True)

    with ExitStack() as ctx:
        tc = ctx.enter_context(tile.TileContext(nc))
        consts = ctx.enter_context(tc.tile_pool(name="consts", bufs=1))
        bigK = ctx.enter_context(tc.tile_pool(name="bigK", bufs=2))
        bigV = ctx.enter_context(tc.tile_pool(name="bigV", bufs=1))
        xtp = ctx.enter_context(tc.tile_pool(name="xtp", bufs=4))
        sb = ctx.enter_context(tc.tile_pool(name="sb", bufs=3))
        sm = ctx.enter_context(tc.tile_pool(name="sm", bufs=3))
        psA = ctx.enter_context(tc.tile_pool(name="psA", bufs=2, space="PSUM"))
        psV = ctx.enter_context(tc.tile_pool(name="psV", bufs=2, space="PSUM"))
        psS = ctx.enter_context(tc.tile_pool(name="psS", bufs=2, space="PSUM"))
        psM = ctx.enter_context(tc.tile_pool(name="psM", bufs=2, space="PSUM"))

        # ---- constants into SBUF
        wk_sb = consts.tile([128, 2, A], dt.bfloat16)   # [d%128, d//128, a]
        wv_sb = consts.tile([128, 2, A], dt.bfloat16)
        nc.sync.dma_start(out=wk_sb, in_=wkT[:, :].rearrange("(c p) a -> p c a", p=128))
        nc.sync.dma_start(out=wv_sb, in_=wvT[:, :].rearrange("(c p) a -> p c a", p=128))
        wq_sb = consts.tile([DI, A], dt.bfloat16)
        nc.sync.dma_start(out=wq_sb, in_=wqT[:, :])
        wker_sb = consts.tile([A, A], dt.bfloat16)
        nc.sync.dma_start(out=wker_sb, in_=wkerT[:, :])
        fw_sb = consts.tile([A, D], dt.bfloat16)
        nc.sync.dma_start(out=fw_sb, in_=fwT[:, :])
        idb_sb = consts.tile([128, 128], dt.bfloat16)
        nc.sync.dma_start(out=idb_sb, in_=idb[:, :])
        idf_sb = consts.tile([128, 128], dt.float32)
        nc.sync.dma_start(out=idf_sb, in_=idf[:, :])
        bias_sb = consts.tile([128, D], dt.float32)
        nc.gpsimd.dma_start(out=bias_sb, in_=fb[:, :].to_broadcast((128, D)))

        for bt in range(NBT):
            b0 = bt * BT

            # ================= Stage A: xT loads + K/V projections ========
            kt_sb = bigK.tile([128, BT * S], dt.bfloat16, tag="kt")
            v_sb = bigV.tile([128, BT * 2 * 128], dt.bfloat16, tag="v")
            PPX = 4                      # batch pairs per xT load tile
            MC = PPX * 2 * S             # 3200 m-columns per load
            for p in range(NPAIR):
                if p % PPX == 0:
                    m0 = b0 * S + p * 2 * S
                    xt0 = xtp.tile([128, MC], dt.bfloat16, tag="xt")
                    xt1 = xtp.tile([128, MC], dt.bfloat16, tag="xt")
                    nc.sync.dma_start(out=xt0, in_=xT[0:128, m0 : m0 + MC])
                    nc.sync.dma_start(out=xt1, in_=xT[128:256, m0 : m0 + MC])
                q0 = (p % PPX) * 2 * S   # this pair's columns inside the tile

                # K^T [a, 400]: accumulate over the two d-chunks
                ps_kt = psA.tile([128, 2 * S], dt.float32, tag="ps")
                nc.tensor.matmul(ps_kt, wk_sb[:, 0, :], xt0[:, q0 : q0 + 2 * S],
                                 start=True, stop=False)
                nc.tensor.matmul(ps_kt, wk_sb[:, 1, :], xt1[:, q0 : q0 + 2 * S],
                                 start=False, stop=True)
                # prelu + cast -> KT columns for this pair
                # (alternate pairs on DVE to unload the ACT-throttled stage A)
                if p % 2 == 0:
                    nc.scalar.activation(
                        out=kt_sb[:, p * 2 * S : (p + 1) * 2 * S],
                        in_=ps_kt, func=AF.Prelu, alpha=alpha)
                else:
                    ktmp = sm.tile([128, 2 * S], dt.bfloat16, tag="ktmp")
                    nc.vector.tensor_scalar_mul(ktmp, ps_kt, alpha)
                    nc.vector.tensor_max(
                        kt_sb[:, p * 2 * S : (p + 1) * 2 * S], ps_kt, ktmp)

                # V [s-chunk rows, a] in 4 blocks (2 b's x 2 s-chunks)
                ps_v = psV.tile([128, 512], dt.float32, tag="ps")
                first, last = (0, 0), (3, 1)
                for j in range(4):
                    bip, sc = divmod(j, 2)
                    cols = q0 + bip * S + SC_OFF[sc]
                    ln = SC_LEN[sc]
                    for c, xt in enumerate((xt0, xt1)):
                        nc.tensor.matmul(
                            ps_v[0:ln, j * 128 : (j + 1) * 128],
                            xt[:, cols : cols + ln],
                            (wv_sb[:, 0, :], wv_sb[:, 1, :])[c],
                            start=(j, c) == first, stop=(j, c) == last)
                nc.scalar.activation(
                    out=v_sb[:, p * 512 : (p + 1) * 512], in_=ps_v,
                    func=AF.Prelu, alpha=alpha)

            # ================= Stage B: Q -> Qt^T [a, b] ==================
            eT = sm.tile([DI, BT], dt.bfloat16, tag="eT")
            nc.sync.dma_start_transpose(eT, eb[b0 : b0 + BT, :])
            ps_q = psM.tile([A, BT], dt.float32, tag="ps")
            nc.tensor.matmul(ps_q, wq_sb, eT, start=True, stop=True)
            qT_sb = sm.tile([A, BT], dt.bfloat16, tag="qT")
            nc.scalar.activation(out=qT_sb, in_=ps_q, func=AF.Prelu, alpha=alpha)
            ps_qt = psM.tile([A, BT], dt.float32, tag="ps")
            nc.tensor.matmul(ps_qt, wker_sb, qT_sb, start=True, stop=True)
            qtT_sb = sm.tile([A, BT], dt.bfloat16, tag="qtT")
            nc.scalar.activation(out=qtT_sb, in_=ps_qt, func=AF.Copy,
                                 scale=1.0 / SCALE)

            # ================= Stage C: scores + softmax ==================
            ps_s0 = psS.tile([128, BT], dt.float32, tag="ps")
            ps_s1 = psS.tile([128, BT], dt.float32, tag="ps")
            for bl in range(BT):
                boff = bl * S
                nc.tensor.matmul(
                    ps_s0[:, bl : bl + 1],
                    kt_sb[:, boff + SC_OFF[0] : boff + SC_OFF[0] + 128],
                    qtT_sb[:, bl : bl + 1],
                    start=(bl == 0), stop=(bl == BT - 1))
                nc.tensor.matmul(
                    ps_s1[0 : SC_LEN[1], bl : bl + 1],
                    kt_sb[:, boff + SC_OFF[1] : boff + S],
                    qtT_sb[:, bl : bl + 1],
                    start=(bl == 0), stop=(bl == BT - 1))
            c0_sb = sm.tile([128, BT], dt.float32, tag="c0")
            c1_sb = sm.tile([128, BT], dt.float32, tag="c1")
            nc.scalar.copy(out=c0_sb, in_=ps_s0)
            nc.scalar.copy(out=c1_sb, in_=ps_s1)
            ps_t = psM.tile([BT, S], dt.float32, tag="ps")
            nc.tensor.transpose(ps_t[:, 0:128], c0_sb, idf_sb)
            nc.tensor.transpose(ps_t[:, 128:S], c1_sb[0 : SC_LEN[1], :], idf_sb[0 : SC_LEN[1], 0 : SC_LEN[1]])

            neg_sb = sm.tile([BT, S], dt.float32, tag="neg")
            nc.sync.dma_start(out=neg_sb, in_=neg[b0 : b0 + BT, :])
            sc_sb = sm.tile([BT, S], dt.float32, tag="sc")
            nc.vector.tensor_add(sc_sb, ps_t, neg_sb)

            nmx = sm.tile([BT, 1], dt.float32, tag="nmx")
            nc.vector.tensor_reduce(out=nmx, in_=sc_sb, axis=mybir.AxisListType.X,
                                    op=OP.max, negate=True)
            p_sb = sm.tile([BT, S], dt.float32, tag="p")
            sum_sb = sm.tile([BT, 1], dt.float32, tag="sum")
            nc.scalar.activation(out=p_sb, in_=sc_sb, func=AF.Exp,
                                 bias=nmx, scale=1.0, accum_out=sum_sb)
            # attn = exp(sc - max - ln(sum)) : normalized softmax in one pass
            ls = sm.tile([BT, 1], dt.float32, tag="ls")
            nc.scalar.activation(out=ls, in_=sum_sb, func=AF.Ln)
            b2 = sm.tile([BT, 1], dt.float32, tag="b2")
            nc.vector.tensor_sub(b2, nmx, ls)
            at_f = sm.tile([BT, S], dt.float32, tag="atf")
            nc.scalar.activation(out=at_f, in_=sc_sb, func=AF.Exp, bias=b2)
            nc.sync.dma_start(out=attn_o[b0 : b0 + BT, :], in_=at_f)
            at_b = sm.tile([BT, S], dt.bfloat16, tag="atb")
            nc.vector.tensor_copy(at_b, at_f)

            # ============ Stage D: attn^T chunks [s, b] (bf16) ============
            ps_a0 = psM.tile([128, BT], dt.bfloat16, tag="ps")
            ps_a1 = psM.tile([128, BT], dt.bfloat16, tag="ps")
            nc.tensor.transpose(ps_a0, at_b[:, 0:128], idb_sb)
            nc.tensor.transpose(ps_a1[0 : SC_LEN[1], :], at_b[:, 128:S], idb_sb)
            aT0 = sm.tile([128, BT], dt.bfloat16, tag="aT0")
            aT1 = sm.tile([SC_LEN[1], BT], dt.bfloat16, tag="aT1")
            nc.vector.tensor_copy(aT0, ps_a0)
            nc.vector.tensor_copy(aT1, ps_a1[0 : SC_LEN[1], :])

            # ============ Stage E: out = attn.V  + ffn ====================
            ps_o = psM.tile([A, BT], dt.float32, tag="ps")
            for bl in range(BT):
                blk = bl * 2
                nc.tensor.matmul(
                    ps_o[:, bl : bl + 1],
                    v_sb[:, blk * 128 : (blk + 1) * 128],
                    aT0[:, bl : bl + 1],
                    start=(bl == 0), stop=False)
                nc.tensor.matmul(
                    ps_o[:, bl : bl + 1],
                    v_sb[0 : SC_LEN[1], (blk + 1) * 128 : (blk + 2) * 128],
                    aT1[:, bl : bl + 1],
                    start=False, stop=(bl == BT - 1))
            oT_sb = sm.tile([A, BT], dt.bfloat16, tag="oT")
            nc.vector.tensor_copy(oT_sb, ps_o)

            ps_f = psM.tile([BT, D], dt.float32, tag="ps")
            nc.tensor.matmul(ps_f, oT_sb, fw_sb, start=True, stop=True)
            f0 = sm.tile([BT, D], dt.float32, tag="f0")
            nc.vector.tensor_add(f0, ps_f, bias_sb)
            f1 = sm.tile([BT, D], dt.float32, tag="f1")
            nc.scalar.activation(out=f1, in_=f0, func=AF.Prelu, alpha=alpha)
            nc.sync.dma_start(out=ffn_o[b0 : b0 + BT, :], in_=f1)

    return nc


def _get_nc(alpha: float):
    key = ("nc", alpha)
    if key not in _cache:
        _cache[key] = _build(alpha)
    return _cache[key]


def kernel(transformer_out, mask, target_item_emb, W_q, W_k, W_v, W_kernel,
           ffn_W, ffn_b, prelu_a):
    from concourse.bass_utils import run_bass_kernel_spmd

    alpha = float(np.asarray(prelu_a))
    nc = _get_nc(alpha)

    x = np.asarray(transformer_out, dtype=np.float32)
    xb = x.astype(BF16).reshape(B * S, D)
    negm = (np.asarray(mask, dtype=np.float32) * np.float32(-1e9))
    e = np.asarray(target_item_emb, dtype=np.float32).astype(BF16)
    shared = {
        "wkT": np.ascontiguousarray(np.asarray(W_k, np.float32).T).astype(BF16),
        "wvT": np.ascontiguousarray(np.asarray(W_v, np.float32).T).astype(BF16),
        "wqT": np.ascontiguousarray(np.asarray(W_q, np.float32).T).astype(BF16),
        "wkerT": np.ascontiguousarray(np.asarray(W_kernel, np.float32).T).astype(BF16),
        "fwT": np.ascontiguousarray(np.asarray(ffn_W, np.float32).T).astype(BF16),
        "fb": np.asarray(ffn_b, np.float32).reshape(1, D),
        "idb": np.eye(128, dtype=np.float32).astype(BF16),
        "idf": np.eye(128, dtype=np.float32),
    }
    in_maps = []
    for c in range(N_CORES):
        bs = c * BC
        in_maps.append({
            "xT": np.ascontiguousarray(xb[bs * S : (bs + BC) * S].T),
            "neg": negm[bs : bs + BC],
            "eb": e[bs : bs + BC],
            **shared,
        })

    global LAST_NC, LAST_IN_MAPS
    LAST_NC, LAST_IN_MAPS = nc, in_maps
    res = run_bass_kernel_spmd(nc, in_maps, core_ids=list(range(N_CORES)))
    ffn_out = np.concatenate([r["ffn_o"] for r in res.results], axis=0)
    attn = np.concatenate([r["attn_o"] for r in res.results], axis=0)
    return ffn_out, attn.reshape(B, 1, S)


LAST_NC = None
LAST_IN_MAPS = None


# revision 17
# speedup vs baseline: 1.0516x; 1.0516x over previous
"""Trainium2 Bass kernel for nn_PoolingLayer (target-attention pooling layer).

Computation (per batch b):
  K = prelu(x @ W_k.T), V = prelu(x @ W_v.T)           x: [S, D]
  Q = prelu(e @ W_q.T);  Qt = W_kernel @ Q
  score[s] = K[s] . Qt / sqrt(A);  masked softmax over s -> attn
  out = sum_s attn[s] * V[s];  ffn_out = prelu(out @ ffn_W.T + ffn_b)
Returns (ffn_out [B, D], attn [B, 1, S]).

Sharding: pure data-parallel over batch across 8 NeuronCores (256 b/core).
On-chip layout: per b-tile of 128 batches; x streamed in via DMA-transpose
(bf16) as xT [d, m] tiles; K kept transposed [a, m] for per-b score matmuls;
V kept row-major in per-(b, s-chunk) blocks for per-b attn.V matmuls, with
s-chunks [0:128] and [72:200] (the 56-column overlap of chunk 1 is zeroed in
the transposed-attention operand so nothing is double counted).
"""

import numpy as np
import ml_dtypes
from contextlib import ExitStack

B, S, D, A = 2048, 200, 256, 128
DI = 64                 # target item embedding dim
N_CORES = 8
BC = B // N_CORES       # 256 batches per core
BT = 128                # b-tile (partition) size
NBT = BC // BT          # b-tiles per core
NPAIR = BT // 2         # batch pairs per b-tile (xT loaded per pair: 400 rows)
SCALE = float(A) ** 0.5
SC_OFF = (0, 128)       # s-chunk starts: [0:128], [128:200]
SC_LEN = (128, S - 128) # s-chunk lengths: 128, 72

BF16 = ml_dtypes.bfloat16

_cache = {}


# ---------------------------------------------------------------------------
# walrus workaround: this build accepts only 1 sem-wait on the Tile kernel-tail
# Drain; split the waits across single-wait sync NoOps.
def _install_tile_patch():
    import bass_rust
    import concourse.tile as _tile
    from concourse.vector_clock import ScopedClock

    if getattr(_tile.TileContext, "_drain_patched", False):
        return

    def _patched(self, tick_clock, wait_clock):
        nc = self.nc
        drain_inst = nc.sync.drain()
        wait_clock.add_sem_waits(
            drain_inst.ins, ScopedClock({None: tick_clock.global_clock})
        )
        si = drain_inst.ins.sync_info
        if si is not None and len(si.on_wait) > 1:
            waits = list(si.on_wait)
            si.on_wait = waits[:1]
            for w in waits[1:]:
                n = nc.sync.nop(nofuse=True)
                n.ins.sync_info = bass_rust.SyncInfo(on_update=[], on_wait=[w])
        nc.all_engine_barrier()
        assert self.sems is not None
        popped = nc._tile_sem_poison_stack.pop()
        assert popped is self._sem_poison
        nc.clear_and_free_semaphores(list(self.sems.allocated().values()))
        nc.all_engine_barrier()

    _tile.TileContext._drain_and_barrier = _patched
    _tile.TileContext._drain_patched = True

    # Global workaround: walrus accepts at most ONE sem-wait per instruction.
    # Rewrite the serialized BIR: move extra waits onto same-engine NoOps
    # inserted immediately before the over-subscribed instruction.
    import orjson
    import concourse.bass as _bass

    if getattr(_bass.Bass, "_json_wait_patched", False):
        return
    _orig_to_json_bytes = _bass.Bass.to_json_bytes

    def _split_waits_json(self):
        raw = _orig_to_json_bytes(self)
        bir = orjson.loads(raw)
        n = [0]

        def fix_block(bb):
            out = []
            for ins in bb.get("instructions", []):
                si = ins.get("sync_info") or {}
                w = si.get("on_wait") or []
                if len(w) > 1:
                    for extra in w[:-1]:
                        n[0] += 1
                        out.append({
                            "debug": ins.get("debug", 0),
                            "engine": ins["engine"],
                            "ins": [], "outs": [],
                            "name": f"{ins['name']}-wsplit{n[0]}",
                            "opcode": "NoOp",
                            "sync_info": {"on_update": [], "on_wait": [extra]},
                        })
                    si["on_wait"] = [w[-1]]
                out.append(ins)
            bb["instructions"] = out
            for sub in bb.get("blocks", []):
                fix_block(sub)

        for fn in bir.get("functions", []):
            for bb in fn.get("blocks", []):
                fix_block(bb)
        return orjson.dumps(bir)

    _bass.Bass.to_json_bytes = _split_waits_json
    _bass.Bass._json_wait_patched = True


# ---------------------------------------------------------------------------
def _build(alpha: float):
    import concourse.bass as bass
    import concourse.mybir as mybir
    import concourse.tile as tile

    _install_tile_patch()
    dt = mybir.dt
    AF = mybir.ActivationFunctionType
    OP = mybir.AluOpType

    nc = bass.Bass()
    xT = nc.declare_dram_parameter("xT", [D, BC * S], dt.bfloat16, isOutput=False)
    neg = nc.declare_dram_parameter("neg", [BC, S], dt.float32, isOutput=False)
    eb = nc.declare_dram_parameter("eb", [BC, DI], dt.bfloat16, isOutput=False)
    wkT = nc.declare_dram_parameter("wkT", [D, A], dt.bfloat16, isOutput=False)
    wvT = nc.declare_dram_parameter("wvT", [D, A], dt.bfloat16, isOutput=False)
    wqT = nc.declare_dram_parameter("wqT", [DI, A], dt.bfloat16, isOutput=False)
    wkerT = nc.declare_dram_parameter("wkerT", [A, A], dt.bfloat16, isOutput=False)
    fwT = nc.declare_dram_parameter("fwT", [A, D], dt.bfloat16, isOutput=False)
    fb = nc.declare_dram_parameter("fb", [1, D], dt.float32, isOutput=False)
    idb = nc.declare_dram_parameter("idb", [128, 128], dt.bfloat16, isOutput=False)
    idf = nc.declare_dram_parameter("idf", [128, 128], dt.float32, isOutput=False)
    ffn_o = nc.declare_dram_parameter("ffn_o", [BC, D], dt.float32, isOutput=True)
    attn_o = nc.declare_dram_parameter("attn_o", [BC, S], dt.float32, isOutput=True)

    with ExitStack() as ctx:
        tc = ctx.enter_context(tile.TileContext(nc))
        consts = ctx.enter_context(tc.tile_pool(name="consts", bufs=1))
        big = ctx.enter_context(tc.tile_pool(name="big", bufs=1))
        xtp = ctx.enter_context(tc.tile_pool(name="xtp", bufs=4))
        sb = ctx.enter_context(tc.tile_pool(name="sb", bufs=3))
        sm = ctx.enter_context(tc.tile_pool(name="sm", bufs=3))
        psA = ctx.enter_context(tc.tile_pool(name="psA", bufs=2, space="PSUM"))
        psV = ctx.enter_context(tc.tile_pool(name="psV", bufs=2, space="PSUM"))
        psS = ctx.enter_context(tc.tile_pool(name="psS", bufs=2, space="PSUM"))
        psM = ctx.enter_context(tc.tile_pool(name="psM", bufs=2, space="PSUM"))

        # ---- constants into SBUF
        wk_sb = consts.tile([128, 2, A], dt.bfloat16)   # [d%128, d//128, a]
        wv_sb = consts.tile([128, 2, A], dt.bfloat16)
        nc.sync.dma_start(out=wk_sb, in_=wkT[:, :].rearrange("(c p) a -> p c a", p=128))
        nc.sync.dma_start(out=wv_sb, in_=wvT[:, :].rearrange("(c p) a -> p c a", p=128))
        wq_sb = consts.tile([DI, A], dt.bfloat16)
        nc.sync.dma_start(out=wq_sb, in_=wqT[:, :])
        wker_sb = consts.tile([A, A], dt.bfloat16)
        nc.sync.dma_start(out=wker_sb, in_=wkerT[:, :])
        fw_sb = consts.tile([A, D], dt.bfloat16)
        nc.sync.dma_start(out=fw_sb, in_=fwT[:, :])
        idb_sb = consts.tile([128, 128], dt.bfloat16)
        nc.sync.dma_start(out=idb_sb, in_=idb[:, :])
        idf_sb = consts.tile([128, 128], dt.float32)
        nc.sync.dma_start(out=idf_sb, in_=idf[:, :])
        bias_sb = consts.tile([128, D], dt.float32)
        nc.gpsimd.dma_start(out=bias_sb, in_=fb[:, :].to_broadcast((128, D)))

        for bt in range(NBT):
            b0 = bt * BT

            # ================= Stage A: xT loads + K/V projections ========
            kt_sb = big.tile([128, BT * S], dt.bfloat16, tag="kt")
            v_sb = big.tile([128, BT * 2 * 128], dt.bfloat16, tag="v")
            PPX = 8                      # batch pairs per xT load tile
            MC = PPX * 2 * S             # 3200 m-columns per load
            for p in range(NPAIR):
                if p % PPX == 0:
                    m0 = b0 * S + p * 2 * S
                    xt0 = xtp.tile([128, MC], dt.bfloat16, tag="xt")
                    xt1 = xtp.tile([128, MC], dt.bfloat16, tag="xt")
                    nc.sync.dma_start(out=xt0, in_=xT[0:128, m0 : m0 + MC])
                    nc.sync.dma_start(out=xt1, in_=xT[128:256, m0 : m0 + MC])
                q0 = (p % PPX) * 2 * S   # this pair's columns inside the tile

                # K^T [a, 400]: accumulate over the two d-chunks
                ps_kt = psA.tile([128, 2 * S], dt.float32, tag="ps")
                nc.tensor.matmul(ps_kt, wk_sb[:, 0, :], xt0[:, q0 : q0 + 2 * S],
                                 start=True, stop=False)
                nc.tensor.matmul(ps_kt, wk_sb[:, 1, :], xt1[:, q0 : q0 + 2 * S],
                                 start=False, stop=True)
                # prelu + cast -> KT columns for this pair
                # (alternate pairs on DVE to unload the ACT-throttled stage A)
                if p % 2 == 0:
                    nc.scalar.activation(
                        out=kt_sb[:, p * 2 * S : (p + 1) * 2 * S],
                        in_=ps_kt, func=AF.Prelu, alpha=alpha)
                else:
                    ktmp = sm.tile([128, 2 * S], dt.float32, tag="ktmp")
                    nc.vector.tensor_scalar_mul(ktmp, ps_kt, alpha)
                    nc.vector.tensor_max(
                        kt_sb[:, p * 2 * S : (p + 1) * 2 * S], ps_kt, ktmp)

                # V [s-chunk rows, a] in 4 blocks (2 b's x 2 s-chunks)
                ps_v = psV.tile([128, 512], dt.float32, tag="ps")
                first, last = (0, 0), (3, 1)
                for j in range(4):
                    bip, sc = divmod(j, 2)
                    cols = q0 + bip * S + SC_OFF[sc]
                    ln = SC_LEN[sc]
                    for c, xt in enumerate((xt0, xt1)):
                        nc.tensor.matmul(
                            ps_v[0:ln, j * 128 : (j + 1) * 128],
                            xt[:, cols : cols + ln],
                            (wv_sb[:, 0, :], wv_sb[:, 1, :])[c],
                            start=(j, c) == first, stop=(j, c) == last)
                nc.scalar.activation(
                    out=v_sb[:, p * 512 : (p + 1) * 512], in_=ps_v,
                    func=AF.Prelu, alpha=alpha)

            # ================= Stage B: Q -> Qt^T [a, b] ==================
            eT = sm.tile([DI, BT], dt.bfloat16, tag="eT")
            nc.sync.dma_start_transpose(eT, eb[b0 : b0 + BT, :])
            ps_q = psM.tile([A, BT], dt.float32, tag="ps")
            nc.tensor.matmul(ps_q, wq_sb, eT, start=True, stop=True)
            qT_sb = sm.tile([A, BT], dt.bfloat16, tag="qT")
            nc.scalar.activation(out=qT_sb, in_=ps_q, func=AF.Prelu, alpha=alpha)
            ps_qt = psM.tile([A, BT], dt.float32, tag="ps")
            nc.tensor.matmul(ps_qt, wker_sb, qT_sb, start=True, stop=True)
            qtT_sb = sm.tile([A, BT], dt.bfloat16, tag="qtT")
            nc.scalar.activation(out=qtT_sb, in_=ps_qt, func=AF.Copy,
                                 scale=1.0 / SCALE)

            # ================= Stage C: scores + softmax ==================
            ps_s0 = psS.tile([128, BT], dt.float32, tag="ps")
            ps_s1 = psS.tile([128, BT], dt.float32, tag="ps")
            for bl in range(BT):
                boff = bl * S
                nc.tensor.matmul(
                    ps_s0[:, bl : bl + 1],
                    kt_sb[:, boff + SC_OFF[0] : boff + SC_OFF[0] + 128],
                    qtT_sb[:, bl : bl + 1],
                    start=(bl == 0), stop=(bl == BT - 1))
                nc.tensor.matmul(
                    ps_s1[0 : SC_LEN[1], bl : bl + 1],
                    kt_sb[:, boff + SC_OFF[1] : boff + S],
                    qtT_sb[:, bl : bl + 1],
                    start=(bl == 0), stop=(bl == BT - 1))
            c0_sb = sm.tile([128, BT], dt.float32, tag="c0")
            c1_sb = sm.tile([128, BT], dt.float32, tag="c1")
            nc.scalar.copy(out=c0_sb, in_=ps_s0)
            nc.scalar.copy(out=c1_sb, in_=ps_s1)
            ps_t = psM.tile([BT, S], dt.float32, tag="ps")
            nc.tensor.transpose(ps_t[:, 0:128], c0_sb, idf_sb)
            nc.tensor.transpose(ps_t[:, 128:S], c1_sb[0 : SC_LEN[1], :], idf_sb[0 : SC_LEN[1], 0 : SC_LEN[1]])

            neg_sb = sm.tile([BT, S], dt.float32, tag="neg")
            nc.sync.dma_start(out=neg_sb, in_=neg[b0 : b0 + BT, :])
            sc_sb = sm.tile([BT, S], dt.float32, tag="sc")
            nc.vector.tensor_add(sc_sb, ps_t, neg_sb)

            nmx = sm.tile([BT, 1], dt.float32, tag="nmx")
            nc.vector.tensor_reduce(out=nmx, in_=sc_sb, axis=mybir.AxisListType.X,
                                    op=OP.max, negate=True)
            p_sb = sm.tile([BT, S], dt.float32, tag="p")
            sum_sb = sm.tile([BT, 1], dt.float32, tag="sum")
            nc.scalar.activation(out=p_sb, in_=sc_sb, func=AF.Exp,
                                 bias=nmx, scale=1.0, accum_out=sum_sb)
            # attn = exp(sc - max - ln(sum)) : normalized softmax in one pass
            ls = sm.tile([BT, 1], dt.float32, tag="ls")
            nc.scalar.activation(out=ls, in_=sum_sb, func=AF.Ln)
            b2 = sm.tile([BT, 1], dt.float32, tag="b2")
            nc.vector.tensor_sub(b2, nmx, ls)
            at_f = sm.tile([BT, S], dt.float32, tag="atf")
            nc.scalar.activation(out=at_f, in_=sc_sb, func=AF.Exp, bias=b2)
            nc.sync.dma_start(out=attn_o[b0 : b0 + BT, :], in_=at_f)
            at_b = sm.tile([BT, S], dt.bfloat16, tag="atb")
            nc.vector.tensor_copy(at_b, at_f)

            # ============ Stage D: attn^T chunks [s, b] (bf16) ============
            ps_a0 = psM.tile([128, BT], dt.bfloat16, tag="ps")
            ps_a1 = psM.tile([128, BT], dt.bfloat16, tag="ps")
            nc.tensor.transpose(ps_a0, at_b[:, 0:128], idb_sb)
            nc.tensor.transpose(ps_a1[0 : SC_LEN[1], :], at_b[:, 128:S], idb_sb)
            aT0 = sm.tile([128, BT], dt.bfloat16, tag="aT0")
            aT1 = sm.tile([SC_LEN[1], BT], dt.bfloat16, tag="aT1")
            nc.vector.tensor_copy(aT0, ps_a0)
            nc.vector.tensor_copy(aT1, ps_a1[0 : SC_LEN[1], :])

            # ============ Stage E: out = attn.V  + ffn ====================
            ps_o = psM.tile([A, BT], dt.float32, tag="ps")
            for bl in range(BT):
                blk = bl * 2
                nc.tensor.matmul(
                    ps_o[:, bl : bl + 1],
                    v_sb[:, blk * 128 : (blk + 1) * 128],
                    aT0[:, bl : bl + 1],
                    start=(bl == 0), stop=False)
                nc.tensor.matmul(
                    ps_o[:, bl : bl + 1],
                    v_sb[0 : SC_LEN[1], (blk + 1) * 128 : (blk + 2) * 128],
                    aT1[:, bl : bl + 1],
                    start=False, stop=(bl == BT - 1))
            oT_sb = sm.tile([A, BT], dt.bfloat16, tag="oT")
            nc.vector.tensor_copy(oT_sb, ps_o)

            ps_f = psM.tile([BT, D], dt.float32, tag="ps")
            nc.tensor.matmul(ps_f, oT_sb, fw_sb, start=True, stop=True)
            f0 = sm.tile([BT, D], dt.float32, tag="f0")
            nc.vector.tensor_add(f0, ps_f, bias_sb)
            f1 = sm.tile([BT, D], dt.float32, tag="f1")
            nc.scalar.activation(out=f1, in_=f0, func=AF.Prelu, alpha=alpha)
            nc.sync.dma_start(out=ffn_o[b0 : b0 + BT, :], in_=f1)

    return nc


def _get_nc(alpha: float):
    key = ("nc", alpha)
    if key not in _cache:
        _cache[key] = _build(alpha)
    return _cache[key]


def kernel(transformer_out, mask, target_item_emb, W_q, W_k, W_v, W_kernel,
           ffn_W, ffn_b, prelu_a):
    from concourse.bass_utils import run_bass_kernel_spmd

    alpha = float(np.asarray(prelu_a))
    nc = _get_nc(alpha)

    x = np.asarray(transformer_out, dtype=np.float32)
    xb = x.astype(BF16).reshape(B * S, D)
    negm = (np.asarray(mask, dtype=np.float32) * np.float32(-1e9))
    e = np.asarray(target_item_emb, dtype=np.float32).astype(BF16)
    shared = {
        "wkT": np.ascontiguousarray(np.asarray(W_k, np.float32).T).astype(BF16),
        "wvT": np.ascontiguousarray(np.asarray(W_v, np.float32).T).astype(BF16),
        "wqT": np.ascontiguousarray(np.asarray(W_q, np.float32).T).astype(BF16),
        "wkerT": np.ascontiguousarray(np.asarray(W_kernel, np.float32).T).astype(BF16),
        "fwT": np.ascontiguousarray(np.asarray(ffn_W, np.float32).T).astype(BF16),
        "fb": np.asarray(ffn_b, np.float32).reshape(1, D),
        "idb": np.eye(128, dtype=np.float32).astype(BF16),
        "idf": np.eye(128, dtype=np.float32),
    }
    in_maps = []
    for c in range(N_CORES):
        bs = c * BC
        in_maps.append({
            "xT": np.ascontiguousarray(xb[bs * S : (bs + BC) * S].T),
            "neg": negm[bs : bs + BC],
            "eb": e[bs : bs + BC],
            **shared,
        })

    global LAST_NC, LAST_IN_MAPS
    LAST_NC, LAST_IN_MAPS = nc, in_maps
    res = run_bass_kernel_spmd(nc, in_maps, core_ids=list(range(N_CORES)))
    ffn_out = np.concatenate([r["ffn_o"] for r in res.results], axis=0)
    attn = np.concatenate([r["attn_o"] for r in res.results], axis=0)
    return ffn_out, attn.reshape(B, 1, S)


LAST_NC = None
LAST_IN_MAPS = None


# revision 18
# speedup vs baseline: 1.0789x; 1.0259x over previous
"""Trainium2 Bass kernel for nn_PoolingLayer (target-attention pooling layer).

Computation (per batch b):
  K = prelu(x @ W_k.T), V = prelu(x @ W_v.T)           x: [S, D]
  Q = prelu(e @ W_q.T);  Qt = W_kernel @ Q
  score[s] = K[s] . Qt / sqrt(A);  masked softmax over s -> attn
  out = sum_s attn[s] * V[s];  ffn_out = prelu(out @ ffn_W.T + ffn_b)
Returns (ffn_out [B, D], attn [B, 1, S]).

Sharding: pure data-parallel over batch across 8 NeuronCores (256 b/core).
On-chip layout: per b-tile of 128 batches; x streamed in via DMA-transpose
(bf16) as xT [d, m] tiles; K kept transposed [a, m] for per-b score matmuls;
V kept row-major in per-(b, s-chunk) blocks for per-b attn.V matmuls, with
s-chunks [0:128] and [72:200] (the 56-column overlap of chunk 1 is zeroed in
the transposed-attention operand so nothing is double counted).
"""

import numpy as np
import ml_dtypes
from contextlib import ExitStack

B, S, D, A = 2048, 200, 256, 128
DI = 64                 # target item embedding dim
N_CORES = 8
BC = B // N_CORES       # 256 batches per core
BT = 128                # b-tile (partition) size
NBT = BC // BT          # b-tiles per core
NPAIR = BT // 2         # batch pairs per b-tile (xT loaded per pair: 400 rows)
SCALE = float(A) ** 0.5
SC_OFF = (0, 128)       # s-chunk starts: [0:128], [128:200]
SC_LEN = (128, S - 128) # s-chunk lengths: 128, 72

BF16 = ml_dtypes.bfloat16

_cache = {}


# ---------------------------------------------------------------------------
# walrus workaround: this build accepts only 1 sem-wait on the Tile kernel-tail
# Drain; split the waits across single-wait sync NoOps.
def _install_tile_patch():
    import bass_rust
    import concourse.tile as _tile
    from concourse.vector_clock import ScopedClock

    if getattr(_tile.TileContext, "_drain_patched", False):
        return

    def _patched(self, tick_clock, wait_clock):
        nc = self.nc
        drain_inst = nc.sync.drain()
        wait_clock.add_sem_waits(
            drain_inst.ins, ScopedClock({None: tick_clock.global_clock})
        )
        si = drain_inst.ins.sync_info
        if si is not None and len(si.on_wait) > 1:
            waits = list(si.on_wait)
            si.on_wait = waits[:1]
            for w in waits[1:]:
                n = nc.sync.nop(nofuse=True)
                n.ins.sync_info = bass_rust.SyncInfo(on_update=[], on_wait=[w])
        nc.all_engine_barrier()
        assert self.sems is not None
        popped = nc._tile_sem_poison_stack.pop()
        assert popped is self._sem_poison
        nc.clear_and_free_semaphores(list(self.sems.allocated().values()))
        nc.all_engine_barrier()

    _tile.TileContext._drain_and_barrier = _patched
    _tile.TileContext._drain_patched = True

    # Global workaround: walrus accepts at most ONE sem-wait per instruction.
    # Rewrite the serialized BIR: move extra waits onto same-engine NoOps
    # inserted immediately before the over-subscribed instruction.
    import orjson
    import concourse.bass as _bass

    if getattr(_bass.Bass, "_json_wait_patched", False):
        return
    _orig_to_json_bytes = _bass.Bass.to_json_bytes

    def _split_waits_json(self):
        raw = _orig_to_json_bytes(self)
        bir = orjson.loads(raw)
        n = [0]

        def fix_block(bb):
            out = []
            for ins in bb.get("instructions", []):
                si = ins.get("sync_info") or {}
                w = si.get("on_wait") or []
                if len(w) > 1:
                    for extra in w[:-1]:
                        n[0] += 1
                        out.append({
                            "debug": ins.get("debug", 0),
                            "engine": ins["engine"],
                            "ins": [], "outs": [],
                            "name": f"{ins['name']}-wsplit{n[0]}",
                            "opcode": "NoOp",
                            "sync_info": {"on_update": [], "on_wait": [extra]},
                        })
                    si["on_wait"] = [w[-1]]
                out.append(ins)
            bb["instructions"] = out
            for sub in bb.get("blocks", []):
                fix_block(sub)

        for fn in bir.get("functions", []):
            for bb in fn.get("blocks", []):
                fix_block(bb)
        return orjson.dumps(bir)

    _bass.Bass.to_json_bytes = _split_waits_json
    _bass.Bass._json_wait_patched = True


# ---------------------------------------------------------------------------
def _build(alpha: float):
    import concourse.bass as bass
    import concourse.mybir as mybir
    import concourse.tile as tile

    _install_tile_patch()
    dt = mybir.dt
    AF = mybir.ActivationFunctionType
    OP = mybir.AluOpType

    nc = bass.Bass()
    xT = nc.declare_dram_parameter("xT", [D, BC * S], dt.bfloat16, isOutput=False)
    neg = nc.declare_dram_parameter("neg", [BC, S], dt.float32, isOutput=False)
    eb = nc.declare_dram_parameter("eb", [BC, DI], dt.bfloat16, isOutput=False)
    wkT = nc.declare_dram_parameter("wkT", [D, A], dt.bfloat16, isOutput=False)
    wvT = nc.declare_dram_parameter("wvT", [D, A], dt.bfloat16, isOutput=False)
    wqT = nc.declare_dram_parameter("wqT", [DI, A], dt.bfloat16, isOutput=False)
    wkerT = nc.declare_dram_parameter("wkerT", [A, A], dt.bfloat16, isOutput=False)
    fwT = nc.declare_dram_parameter("fwT", [A, D], dt.bfloat16, isOutput=False)
    fb = nc.declare_dram_parameter("fb", [1, D], dt.float32, isOutput=False)
    idb = nc.declare_dram_parameter("idb", [128, 128], dt.bfloat16, isOutput=False)
    idf = nc.declare_dram_parameter("idf", [128, 128], dt.float32, isOutput=False)
    ffn_o = nc.declare_dram_parameter("ffn_o", [BC, D], dt.float32, isOutput=True)
    attn_o = nc.declare_dram_parameter("attn_o", [BC, S], dt.float32, isOutput=True)

    with ExitStack() as ctx:
        tc = ctx.enter_context(tile.TileContext(nc))
        consts = ctx.enter_context(tc.tile_pool(name="consts", bufs=1))
        big = ctx.enter_context(tc.tile_pool(name="big", bufs=1))
        xtp = ctx.enter_context(tc.tile_pool(name="xtp", bufs=4))
        sb = ctx.enter_context(tc.tile_pool(name="sb", bufs=3))
        sm = ctx.enter_context(tc.tile_pool(name="sm", bufs=3))
        psA = ctx.enter_context(tc.tile_pool(name="psA", bufs=2, space="PSUM"))
        psV = ctx.enter_context(tc.tile_pool(name="psV", bufs=2, space="PSUM"))
        psS = ctx.enter_context(tc.tile_pool(name="psS", bufs=2, space="PSUM"))
        psM = ctx.enter_context(tc.tile_pool(name="psM", bufs=2, space="PSUM"))

        # ---- constants into SBUF
        wk_sb = consts.tile([128, 2, A], dt.bfloat16)   # [d%128, d//128, a]
        wv_sb = consts.tile([128, 2, A], dt.bfloat16)
        nc.sync.dma_start(out=wk_sb, in_=wkT[:, :].rearrange("(c p) a -> p c a", p=128))
        nc.sync.dma_start(out=wv_sb, in_=wvT[:, :].rearrange("(c p) a -> p c a", p=128))
        wq_sb = consts.tile([DI, A], dt.bfloat16)
        nc.sync.dma_start(out=wq_sb, in_=wqT[:, :])
        wker_sb = consts.tile([A, A], dt.bfloat16)
        nc.sync.dma_start(out=wker_sb, in_=wkerT[:, :])
        fw_sb = consts.tile([A, D], dt.bfloat16)
        nc.sync.dma_start(out=fw_sb, in_=fwT[:, :])
        idb_sb = consts.tile([128, 128], dt.bfloat16)
        nc.sync.dma_start(out=idb_sb, in_=idb[:, :])
        idf_sb = consts.tile([128, 128], dt.float32)
        nc.sync.dma_start(out=idf_sb, in_=idf[:, :])
        bias_sb = consts.tile([128, D], dt.float32)
        nc.gpsimd.dma_start(out=bias_sb, in_=fb[:, :].to_broadcast((128, D)))

        for bt in range(NBT):
            b0 = bt * BT

            # ================= Stage A: xT loads + K/V projections ========
            kt_sb = big.tile([128, BT * S], dt.bfloat16, tag="kt")
            v_sb = big.tile([128, BT * 2 * 128], dt.bfloat16, tag="v")
            PPX = 8                      # batch pairs per xT load tile
            MC = PPX * 2 * S             # 3200 m-columns per load
            for p in range(NPAIR):
                if p % PPX == 0:
                    m0 = b0 * S + p * 2 * S
                    xt0 = xtp.tile([128, MC], dt.bfloat16, tag="xt")
                    xt1 = xtp.tile([128, MC], dt.bfloat16, tag="xt")
                    nc.sync.dma_start(out=xt0, in_=xT[0:128, m0 : m0 + MC])
                    nc.sync.dma_start(out=xt1, in_=xT[128:256, m0 : m0 + MC])
                q0 = (p % PPX) * 2 * S   # this pair's columns inside the tile

                # K^T [a, 400]: accumulate over the two d-chunks
                ps_kt = psA.tile([128, 2 * S], dt.float32, tag="ps")
                nc.tensor.matmul(ps_kt, wk_sb[:, 0, :], xt0[:, q0 : q0 + 2 * S],
                                 start=True, stop=False)
                nc.tensor.matmul(ps_kt, wk_sb[:, 1, :], xt1[:, q0 : q0 + 2 * S],
                                 start=False, stop=True)
                # prelu + cast -> KT columns for this pair
                # (alternate pairs on DVE to unload the ACT-throttled stage A)
                if p % 3 == 0:
                    nc.scalar.activation(
                        out=kt_sb[:, p * 2 * S : (p + 1) * 2 * S],
                        in_=ps_kt, func=AF.Prelu, alpha=alpha)
                else:
                    ktmp = sm.tile([128, 2 * S], dt.float32, tag="ktmp")
                    nc.vector.tensor_scalar_mul(ktmp, ps_kt, alpha)
                    nc.vector.tensor_max(
                        kt_sb[:, p * 2 * S : (p + 1) * 2 * S], ps_kt, ktmp)

                # V [s-chunk rows, a] in 4 blocks (2 b's x 2 s-chunks)
                ps_v = psV.tile([128, 512], dt.float32, tag="ps")
                first, last = (0, 0), (3, 1)
                for j in range(4):
                    bip, sc = divmod(j, 2)
                    cols = q0 + bip * S + SC_OFF[sc]
                    ln = SC_LEN[sc]
                    for c, xt in enumerate((xt0, xt1)):
                        nc.tensor.matmul(
                            ps_v[0:ln, j * 128 : (j + 1) * 128],
                            xt[:, cols : cols + ln],
                            (wv_sb[:, 0, :], wv_sb[:, 1, :])[c],
                            start=(j, c) == first, stop=(j, c) == last)
                nc.scalar.activation(
                    out=v_sb[:, p * 512 : (p + 1) * 512], in_=ps_v,
                    func=AF.Prelu, alpha=alpha)

            # ================= Stage B: Q -> Qt^T [a, b] ==================
            eT = sm.tile([DI, BT], dt.bfloat16, tag="eT")
            nc.sync.dma_start_transpose(eT, eb[b0 : b0 + BT, :])
            ps_q = psM.tile([A, BT], dt.float32, tag="ps")
            nc.tensor.matmul(ps_q, wq_sb, eT, start=True, stop=True)
            qT_sb = sm.tile([A, BT], dt.bfloat16, tag="qT")
            nc.scalar.activation(out=qT_sb, in_=ps_q, func=AF.Prelu, alpha=alpha)
            ps_qt = psM.tile([A, BT], dt.float32, tag="ps")
            nc.tensor.matmul(ps_qt, wker_sb, qT_sb, start=True, stop=True)
            qtT_sb = sm.tile([A, BT], dt.bfloat16, tag="qtT")
            nc.scalar.activation(out=qtT_sb, in_=ps_qt, func=AF.Copy,
                                 scale=1.0 / SCALE)

            # ================= Stage C: scores + softmax ==================
            ps_s0 = psS.tile([128, BT], dt.float32, tag="ps")
            ps_s1 = psS.tile([128, BT], dt.float32, tag="ps")
            for bl in range(BT):
                boff = bl * S
                nc.tensor.matmul(
                    ps_s0[:, bl : bl + 1],
                    kt_sb[:, boff + SC_OFF[0] : boff + SC_OFF[0] + 128],
                    qtT_sb[:, bl : bl + 1],
                    start=(bl == 0), stop=(bl == BT - 1))
                nc.tensor.matmul(
                    ps_s1[0 : SC_LEN[1], bl : bl + 1],
                    kt_sb[:, boff + SC_OFF[1] : boff + S],
                    qtT_sb[:, bl : bl + 1],
                    start=(bl == 0), stop=(bl == BT - 1))
            c0_sb = sm.tile([128, BT], dt.float32, tag="c0")
            c1_sb = sm.tile([128, BT], dt.float32, tag="c1")
            nc.scalar.copy(out=c0_sb, in_=ps_s0)
            nc.scalar.copy(out=c1_sb, in_=ps_s1)
            ps_t = psM.tile([BT, S], dt.float32, tag="ps")
            nc.tensor.transpose(ps_t[:, 0:128], c0_sb, idf_sb)
            nc.tensor.transpose(ps_t[:, 128:S], c1_sb[0 : SC_LEN[1], :], idf_sb[0 : SC_LEN[1], 0 : SC_LEN[1]])

            neg_sb = sm.tile([BT, S], dt.float32, tag="neg")
            nc.sync.dma_start(out=neg_sb, in_=neg[b0 : b0 + BT, :])
            sc_sb = sm.tile([BT, S], dt.float32, tag="sc")
            nc.vector.tensor_add(sc_sb, ps_t, neg_sb)

            nmx = sm.tile([BT, 1], dt.float32, tag="nmx")
            nc.vector.tensor_reduce(out=nmx, in_=sc_sb, axis=mybir.AxisListType.X,
                                    op=OP.max, negate=True)
            p_sb = sm.tile([BT, S], dt.float32, tag="p")
            sum_sb = sm.tile([BT, 1], dt.float32, tag="sum")
            nc.scalar.activation(out=p_sb, in_=sc_sb, func=AF.Exp,
                                 bias=nmx, scale=1.0, accum_out=sum_sb)
            # attn = exp(sc - max - ln(sum)) : normalized softmax in one pass
            ls = sm.tile([BT, 1], dt.float32, tag="ls")
            nc.scalar.activation(out=ls, in_=sum_sb, func=AF.Ln)
            b2 = sm.tile([BT, 1], dt.float32, tag="b2")
            nc.vector.tensor_sub(b2, nmx, ls)
            at_f = sm.tile([BT, S], dt.float32, tag="atf")
            nc.scalar.activation(out=at_f, in_=sc_sb, func=AF.Exp, bias=b2)
            nc.sync.dma_start(out=attn_o[b0 : b0 + BT, :], in_=at_f)
            at_b = sm.tile([BT, S], dt.bfloat16, tag="atb")
            nc.vector.tensor_copy(at_b, at_f)

            # ============ Stage D: attn^T chunks [s, b] (bf16) ============
            ps_a0 = psM.tile([128, BT], dt.bfloat16, tag="ps")
            ps_a1 = psM.tile([128, BT], dt.bfloat16, tag="ps")
            nc.tensor.transpose(ps_a0, at_b[:, 0:128], idb_sb)
            nc.tensor.transpose(ps_a1[0 : SC_LEN[1], :], at_b[:, 128:S], idb_sb)
            aT0 = sm.tile([128, BT], dt.bfloat16, tag="aT0")
            aT1 = sm.tile([SC_LEN[1], BT], dt.bfloat16, tag="aT1")
            nc.vector.tensor_copy(aT0, ps_a0)
            nc.vector.tensor_copy(aT1, ps_a1[0 : SC_LEN[1], :])

            # ============ Stage E: out = attn.V  + ffn ====================
            ps_o = psM.tile([A, BT], dt.float32, tag="ps")
            for bl in range(BT):
                blk = bl * 2
                nc.tensor.matmul(
                    ps_o[:, bl : bl + 1],
                    v_sb[:, blk * 128 : (blk + 1) * 128],
                    aT0[:, bl : bl + 1],
                    start=(bl == 0), stop=False)
                nc.tensor.matmul(
                    ps_o[:, bl : bl + 1],
                    v_sb[0 : SC_LEN[1], (blk + 1) * 128 : (blk + 2) * 128],
                    aT1[:, bl : bl + 1],
                    start=False, stop=(bl == BT - 1))
            oT_sb = sm.tile([A, BT], dt.bfloat16, tag="oT")
            nc.vector.tensor_copy(oT_sb, ps_o)

            ps_f = psM.tile([BT, D], dt.float32, tag="ps")
            nc.tensor.matmul(ps_f, oT_sb, fw_sb, start=True, stop=True)
            f0 = sm.tile([BT, D], dt.float32, tag="f0")
            nc.vector.tensor_add(f0, ps_f, bias_sb)
            f1 = sm.tile([BT, D], dt.float32, tag="f1")
            nc.scalar.activation(out=f1, in_=f0, func=AF.Prelu, alpha=alpha)
            nc.sync.dma_start(out=ffn_o[b0 : b0 + BT, :], in_=f1)

    return nc


def _get_nc(alpha: float):
    key = ("nc", alpha)
    if key not in _cache:
        _cache[key] = _build(alpha)
    return _cache[key]


def kernel(transformer_out, mask, target_item_emb, W_q, W_k, W_v, W_kernel,
           ffn_W, ffn_b, prelu_a):
    from concourse.bass_utils import run_bass_kernel_spmd

    alpha = float(np.asarray(prelu_a))
    nc = _get_nc(alpha)

    x = np.asarray(transformer_out, dtype=np.float32)
    xb = x.astype(BF16).reshape(B * S, D)
    negm = (np.asarray(mask, dtype=np.float32) * np.float32(-1e9))
    e = np.asarray(target_item_emb, dtype=np.float32).astype(BF16)
    shared = {
        "wkT": np.ascontiguousarray(np.asarray(W_k, np.float32).T).astype(BF16),
        "wvT": np.ascontiguousarray(np.asarray(W_v, np.float32).T).astype(BF16),
        "wqT": np.ascontiguousarray(np.asarray(W_q, np.float32).T).astype(BF16),
        "wkerT": np.ascontiguousarray(np.asarray(W_kernel, np.float32).T).astype(BF16),
        "fwT": np.ascontiguousarray(np.asarray(ffn_W, np.float32).T).astype(BF16),
        "fb": np.asarray(ffn_b, np.float32).reshape(1, D),
        "idb": np.eye(128, dtype=np.float32).astype(BF16),
        "idf": np.eye(128, dtype=np.float32),
    }
    in_maps = []
    for c in range(N_CORES):
        bs = c * BC
        in_maps.append({
            "xT": np.ascontiguousarray(xb[bs * S : (bs + BC) * S].T),
            "neg": negm[bs : bs + BC],
            "eb": e[bs : bs + BC],
            **shared,
        })

    global LAST_NC, LAST_IN_MAPS
    LAST_NC, LAST_IN_MAPS = nc, in_maps
    res = run_bass_kernel_spmd(nc, in_maps, core_ids=list(range(N_CORES)))
    ffn_out = np.concatenate([r["ffn_o"] for r in res.results], axis=0)
    attn = np.concatenate([r["attn_o"] for r in res.results], axis=0)
    return ffn_out, attn.reshape(B, 1, S)


LAST_NC = None
LAST_IN_MAPS = None
